# revision 19
# baseline (speedup 1.0000x reference)
"""LocationAttention TRN2 Bass kernel — data-parallel over batch on 8 NeuronCores.

Reference computation (per batch b):
  proj_enc = enc @ W_enc.T + b_enc                  # [T, A]
  conv     = conv1d(attn_state, W_conv, same pad)   # [C, T]
  attn     = (W_attn @ conv).T                      # [T, A]
  dec      = W_dec @ dec_h                          # [A]
  s        = tanh(attn + proj_enc + dec)            # [T, A]
  out      = s @ W_out[0] + b_out                   # [T]
  w        = softmax(2 * out_masked)                # [T]
  c        = w @ enc                                # [E]

Device mapping (per core, 2 batches):
  - conv+attn fused on host into W_fused = W_attn @ W_conv.flat -> one k=64 matmul
    against sliding windows of attn_state (built by overlapping-window DMA).
  - enc loaded once in natural [t,e] layout; PE-transposed (f32) into [e,t] bf16
    tiles for the projection matmul; natural tiles reused (fp32r) for c.
  - scores computed directly in column form ([128t,1] psum) so exp/softmax and
    the c-matmul weights need no transposes.
  - softmax uses exp(2*out) without max-subtraction (scores are tanh-bounded:
    |2*out| <= 2*sum|W_out| ~ 40, far inside f32 exp range); b_out and the
    constant shift cancel in both w and c.
"""

import sys

sys.path.insert(0, "/opt/trn_rl_repo")

from contextlib import ExitStack

import ml_dtypes
import numpy as np

import concourse.bacc as bacc
import concourse.bass as bass
import concourse.tile as tile
from concourse import mybir
from concourse.bass import ts
from concourse.bass_utils import run_bass_kernel_spmd
from concourse.masks import make_identity

N_CORES = 8
B, T, E, D, A, C = 16, 2048, 1024, 1024, 512, 64
K_STATE, CONV_K = 2, 15
KK = 2 * CONV_K + 1  # 31
TPAD = T + 2 * CONV_K  # 2078
BL = B // N_CORES  # batches per core
NA = A // 128  # 4 a-chunks
NE = E // 128  # 8 e-chunks
TT = 512  # t-tile (free dim of proj matmuls)
NTT = T // TT  # 4 per batch
NTC = T // 128  # 16 t-chunks of 128 per batch

F32 = mybir.dt.float32
F32R = mybir.dt.float32r
BF16 = mybir.dt.bfloat16
AFT = mybir.ActivationFunctionType

_CACHE = {}


def _declare_io(nc):
    t = lambda name, shape, dt, kind: nc.dram_tensor(name, shape, dt, kind=kind).ap()
    io = {
        "enc": t("enc", [BL, T, E], F32, "ExternalInput"),
        "attnpad": t("attnpad", [BL, K_STATE, TPAD], BF16, "ExternalInput"),
        "keep": t("keep", [BL, T], F32, "ExternalInput"),
        "wencT": t("wencT", [E, A], BF16, "ExternalInput"),
        "wfusedT": t("wfusedT", [64, A], BF16, "ExternalInput"),
        "wdecT": t("wdecT", [D, A], BF16, "ExternalInput"),
        "dechT": t("dechT", [D, BL], BF16, "ExternalInput"),
        "bencrow": t("bencrow", [1, A], BF16, "ExternalInput"),
        "woutcol": t("woutcol", [A, 1], BF16, "ExternalInput"),
        "c_out": t("c_out", [BL, E], F32, "ExternalOutput"),
        "w_out": t("w_out", [BL, T], F32, "ExternalOutput"),
    }
    return type("IO", (), io)


def _emit(ctx: ExitStack, tc: tile.TileContext, io, reps=1):
    nc = tc.nc

    const = ctx.enter_context(tc.tile_pool(name="const", bufs=1))
    spsum = ctx.enter_context(tc.tile_pool(name="spsum", bufs=2, space="PSUM"))
    tpsum = ctx.enter_context(tc.tile_pool(name="tpsum", bufs=2, space="PSUM"))
    ppsum = ctx.enter_context(tc.tile_pool(name="ppsum", bufs=2, space="PSUM"))
    cpsum = ctx.enter_context(tc.tile_pool(name="cpsum", bufs=2, space="PSUM"))
    natp = ctx.enter_context(tc.tile_pool(name="natp", bufs=10))
    encTp = ctx.enter_context(tc.tile_pool(name="encTp", bufs=16))
    sp = ctx.enter_context(tc.tile_pool(name="sp", bufs=10))
    shiftp = ctx.enter_context(tc.tile_pool(name="shiftp", bufs=2))
    misc = ctx.enter_context(tc.tile_pool(name="misc", bufs=2))

    # --- constants / weights ---
    ident_f32 = const.tile([128, 128], F32)
    make_identity(nc, ident_f32[:])
    ident = const.tile([128, 128], BF16)
    nc.vector.tensor_copy(ident[:], ident_f32[:])
    ones_mat = const.tile([128, 128], F32)
    nc.vector.memset(ones_mat[:], 1.0)
    ones_1xb = const.tile([1, BL], BF16)
    nc.vector.memset(ones_1xb[:], 1.0)

    wenc = const.tile([128, NE, A], BF16)
    nc.sync.dma_start(wenc[:], io.wencT.rearrange("(c p) a -> p c a", p=128))
    wfused = const.tile([64, A], BF16)
    nc.sync.dma_start(wfused[:], io.wfusedT)
    wdec = const.tile([128, NE, A], BF16)
    nc.sync.dma_start(wdec[:], io.wdecT.rearrange("(c p) a -> p c a", p=128))
    dech = const.tile([128, NE, BL], BF16)
    nc.sync.dma_start(dech[:], io.dechT.rearrange("(c p) b -> p c b", p=128))
    benc = const.tile([1, A], BF16)
    nc.sync.dma_start(benc[:], io.bencrow)
    wout = const.tile([128, NA], BF16)
    nc.sync.dma_start(wout[:], io.woutcol.rearrange("(c p) o -> p (c o)", p=128))
    keepc = const.tile([128, BL, NTC], F32)
    nc.sync.dma_start(keepc[:], io.keep.rearrange("b (c p) -> p b c", p=128))

    # --- per-(batch, a-chunk) tanh bias: dec_h @ W_dec.T + b_enc, in column form ---
    biasc = const.tile([128, NA, BL], F32)
    for ac in range(NA):
        ps = spsum.tile([128, BL], F32, tag="small")
        for ec in range(NE):
            nc.tensor.matmul(
                ps[:], wdec[:, ec, ts(ac, 128)], dech[:, ec, :],
                start=(ec == 0), stop=False,
            )
        nc.tensor.matmul(ps[:], benc[:, ts(ac, 128)], ones_1xb[:], start=False, stop=True)
        nc.scalar.copy(biasc[:, ac, :], ps[:])

    # --- software-pipelined main loop over (batch, t-tile) ---
    # Per iteration i the PE stream is:
    #   transposes(i) | scores(i-1) | proj(i) | c-MMs(i-1)
    # so PE never waits on the ACT/DVE exp->mask->cast chain of its own tile.
    for r in range(reps):
        _emit_rep(nc, io, f"r{r}_", spsum, tpsum, ppsum, cpsum, natp, encTp, sp,
                  shiftp, misc, ident, ones_mat, wenc, wfused, wout, keepc, biasc)


def _emit_rep(nc, io, pfx, spsum, tpsum, ppsum, cpsum, natp, encTp, sp,
              shiftp, misc, ident, ones_mat, wenc, wfused, wout, keepc, biasc):
    batch_state = {}

    def new_batch(b):
        shifted = shiftp.tile([64, T], BF16, tag="shifted", name=f"{pfx}shifted_{b}")
        nc.gpsimd.memset(shifted[:], 0.0)
        for k in range(K_STATE):
            base = io.attnpad[b, k]
            win = bass.AP(base.tensor, base.offset, [[1, KK], [1, T]])
            nc.sync.dma_start(shifted[k * 32 : k * 32 + KK, :], win)
        st = {
            "shifted": shifted,
            "ucol": misc.tile([128, NTC], F32, tag="ucol", name=f"{pfx}ucol_{b}"),
            "ucol_bf": misc.tile([128, NTC], BF16, tag="ucolbf", name=f"{pfx}ucolbf_{b}"),
            "cps": [
                cpsum.tile([1, 512], F32, tag="cps", name=f"{pfx}cps_{b}_{eh}")
                for eh in range(2)
            ],
        }
        batch_state[b] = st
        return st

    def emit_scores(p):
        b, tt, s_tiles, nat_bf = p
        st = batch_state[b]
        for q in range(4):
            ci = tt * 4 + q
            psu = spsum.tile([128, 1], F32, tag="small", name=f"{pfx}psu_{b}_{ci}")
            for ac in range(NA):
                nc.tensor.matmul(
                    psu[:], s_tiles[ac][:, ts(q, 128)], wout[:, ac : ac + 1],
                    start=(ac == 0), stop=(ac == NA - 1),
                )
            nc.scalar.activation(st["ucol"][:, ci : ci + 1], psu[:], AFT.Exp, scale=2.0)
            nc.vector.tensor_mul(
                st["ucol"][:, ci : ci + 1],
                st["ucol"][:, ci : ci + 1],
                keepc[:, b, ci : ci + 1],
            )
            nc.vector.tensor_copy(st["ucol_bf"][:, ci : ci + 1], st["ucol"][:, ci : ci + 1])

    def emit_cmms(p):
        b, tt, s_tiles, nat_bf = p
        st = batch_state[b]
        for q in range(4):
            ci = tt * 4 + q
            for eh in range(2):
                nc.tensor.matmul(
                    st["cps"][eh][:],
                    st["ucol_bf"][:, ci : ci + 1],
                    nat_bf[q][:, ts(eh, 512)],
                    start=(ci == 0), stop=(ci == NTC - 1),
                )

    def emit_batch_tail(b):
        st = batch_state[b]
        usum = misc.tile([128, 1], F32, tag="usum", name=f"{pfx}usum_{b}")
        nc.vector.reduce_sum(usum[:], st["ucol"][:], axis=mybir.AxisListType.X)
        tot = spsum.tile([128, 1], F32, tag="small", name=f"{pfx}tot_{b}")
        nc.tensor.matmul(tot[:], ones_mat[:], usum[:], start=True, stop=True)
        inv = misc.tile([128, 1], F32, tag="inv", name=f"{pfx}inv_{b}")
        nc.vector.reciprocal(inv[:], tot[:])
        wcol = misc.tile([128, NTC], F32, tag="wcol", name=f"{pfx}wcol_{b}")
        nc.vector.tensor_scalar_mul(wcol[:], st["ucol"][:], inv[:])
        nc.sync.dma_start(io.w_out[b].rearrange("(c p) -> p c", p=128), wcol[:])
        crow = misc.tile([1, E], F32, tag="crow", name=f"{pfx}crow_{b}")
        for eh in range(2):
            nc.scalar.activation(
                crow[:, ts(eh, 512)], st["cps"][eh][:], AFT.Copy, scale=inv[0:1, :]
            )
        nc.sync.dma_start(io.c_out[b].unsqueeze(0), crow[:])

    pending = None
    for b in range(BL):
        st = new_batch(b)
        for tt in range(NTT):
            # stage 1: load + cast + transposes of tile i
            nat = []
            nat_bf = []
            for q in range(4):
                nt = natp.tile([128, E], F32, tag="nat", name=f"{pfx}nat_{b}_{tt}_{q}")
                nc.sync.dma_start(
                    nt[:], io.enc[b, tt * TT + q * 128 : tt * TT + (q + 1) * 128, :]
                )
                nat.append(nt)
                nb = natp.tile([128, E], BF16, tag="natbf", name=f"{pfx}natbf_{b}_{tt}_{q}")
                if q % 2 == 0:
                    nc.vector.tensor_copy(nb[:], nt[:])
                else:
                    nc.scalar.copy(nb[:], nt[:])
                nat_bf.append(nb)
            encT = []
            for ec in range(NE):
                pst = tpsum.tile([128, TT], BF16, tag="tr", name=f"{pfx}tr_{b}_{tt}_{ec}")
                for q in range(4):
                    nc.tensor.transpose(
                        pst[:, ts(q, 128)], nat_bf[q][:, ts(ec, 128)], ident[:]
                    )
                eb = encTp.tile([128, TT], BF16, tag="encT", name=f"{pfx}encT_{b}_{tt}_{ec}")
                if ec % 2 == 0:
                    nc.scalar.copy(eb[:], pst[:])
                else:
                    nc.vector.tensor_copy(eb[:], pst[:])
                encT.append(eb)
            # stage 2: scores of tile i-1 (PE work is tiny; feeds ACT/DVE chain)
            if pending is not None:
                emit_scores(pending)
            # stage 3: proj + tanh of tile i
            s_tiles = []
            for ac in range(NA):
                ps = ppsum.tile([128, TT], F32, tag="proj", name=f"{pfx}proj_{b}_{tt}_{ac}")
                for ec in range(NE):
                    nc.tensor.matmul(
                        ps[:], wenc[:, ec, ts(ac, 128)], encT[ec][:],
                        start=(ec == 0), stop=False,
                    )
                nc.tensor.matmul(
                    ps[:], wfused[:, ts(ac, 128)], st["shifted"][:, ts(tt, TT)],
                    start=False, stop=True,
                )
                s = sp.tile([128, TT], BF16, tag="s", name=f"{pfx}s_{b}_{tt}_{ac}")
                nc.scalar.activation(s[:], ps[:], AFT.Tanh, bias=biasc[:, ac, b : b + 1])
                s_tiles.append(s)
            # stage 4: c-matmuls of tile i-1, then its batch tail if it was last
            if pending is not None:
                emit_cmms(pending)
                pb, ptt = pending[0], pending[1]
                if ptt == NTT - 1:
                    emit_batch_tail(pb)
            pending = (b, tt, s_tiles, nat_bf)
    # flush
    emit_scores(pending)
    emit_cmms(pending)
    emit_batch_tail(pending[0])


def build(reps=1):
    key = ("nc", reps)
    if key in _CACHE:
        return _CACHE[key]
    nc = bacc.Bacc("TRN2", target_bir_lowering=False, debug=False, num_devices=N_CORES)
    io = _declare_io(nc)
    with tile.TileContext(nc) as tc, ExitStack() as ctx:
        _emit(ctx, tc, io, reps=reps)
    nc.compile()
    _CACHE[key] = nc
    return nc


def prep_in_maps(inputs):
    enc = np.ascontiguousarray(np.asarray(inputs["encoder_out"], dtype=np.float32))
    mask = np.asarray(inputs["encoder_padding_mask"])
    dec_h = np.asarray(inputs["decoder_h"], dtype=np.float32)
    attn_state = np.asarray(inputs["attn_state"], dtype=np.float32)
    W_enc = np.asarray(inputs["W_enc"], dtype=np.float32)
    b_enc = np.asarray(inputs["b_enc"], dtype=np.float32)
    W_dec = np.asarray(inputs["W_dec"], dtype=np.float32)
    W_attn = np.asarray(inputs["W_attn"], dtype=np.float32)
    W_conv = np.asarray(inputs["W_conv"], dtype=np.float32)
    W_out = np.asarray(inputs["W_out"], dtype=np.float32)

    bf = ml_dtypes.bfloat16
    keep = 1.0 - mask.astype(np.float32)  # [B, T]
    attnpad = np.zeros((B, K_STATE, TPAD), dtype=np.float32)
    attnpad[:, :, CONV_K : CONV_K + T] = attn_state
    attnpad = attnpad.astype(bf)
    # fused conv+attn projection: [A, C] @ [C, K*KK] -> [A, 62] -> pad to 64 rows
    wfu = (W_attn @ W_conv.reshape(C, K_STATE * KK)).reshape(A, K_STATE, KK)
    wfusedT = np.zeros((64, A), dtype=np.float32)
    for k in range(K_STATE):
        wfusedT[k * 32 : k * 32 + KK, :] = wfu[:, k, :].T
    wfusedT = np.ascontiguousarray(wfusedT).astype(bf)
    wencT = np.ascontiguousarray(W_enc.T).astype(bf)
    wdecT = np.ascontiguousarray(W_dec.T).astype(bf)
    bencrow = b_enc.reshape(1, A).astype(bf)
    woutcol = np.ascontiguousarray(W_out.reshape(1, A).T).astype(bf)

    in_maps = []
    for i in range(N_CORES):
        sl = slice(i * BL, (i + 1) * BL)
        in_maps.append(
            {
                "enc": enc[sl],
                "attnpad": np.ascontiguousarray(attnpad[sl]),
                "keep": np.ascontiguousarray(keep[sl]),
                "wencT": wencT,
                "wfusedT": wfusedT,
                "wdecT": wdecT,
                "dechT": np.ascontiguousarray(dec_h[sl].T).astype(bf),
                "bencrow": bencrow,
                "woutcol": woutcol,
            }
        )
    return in_maps


def kernel(**inputs):
    nc = build()
    in_maps = prep_in_maps(inputs)
    res = run_bass_kernel_spmd(nc, in_maps, list(range(N_CORES)))
    c = np.concatenate([res.results[i]["c_out"] for i in range(N_CORES)], axis=0)
    w = np.concatenate([res.results[i]["w_out"] for i in range(N_CORES)], axis=0)
    return c.astype(np.float32), w.astype(np.float32)


# revision 25
# speedup vs baseline: 1.2226x; 1.2226x over previous
"""LocationAttention TRN2 Bass kernel — data-parallel over batch on 8 NeuronCores.

Reference computation (per batch b):
  proj_enc = enc @ W_enc.T + b_enc                  # [T, A]
  conv     = conv1d(attn_state, W_conv, same pad)   # [C, T]
  attn     = (W_attn @ conv).T                      # [T, A]
  dec      = W_dec @ dec_h                          # [A]
  s        = tanh(attn + proj_enc + dec)            # [T, A]
  out      = s @ W_out[0] + b_out                   # [T]
  w        = softmax(2 * out_masked)                # [T]
  c        = w @ enc                                # [E]

Device mapping (per core, 2 batches):
  - conv+attn fused on host into W_fused = W_attn @ W_conv.flat -> one k=64 matmul
    against sliding windows of attn_state (built by overlapping-window DMA).
  - enc loaded once in natural [t,e] layout; PE-transposed (f32) into [e,t] bf16
    tiles for the projection matmul; natural tiles reused (fp32r) for c.
  - scores computed directly in column form ([128t,1] psum) so exp/softmax and
    the c-matmul weights need no transposes.
  - softmax uses exp(2*out) without max-subtraction (scores are tanh-bounded:
    |2*out| <= 2*sum|W_out| ~ 40, far inside f32 exp range); b_out and the
    constant shift cancel in both w and c.
"""

import sys

sys.path.insert(0, "/opt/trn_rl_repo")

from contextlib import ExitStack

import ml_dtypes
import numpy as np

import concourse.bacc as bacc
import concourse.bass as bass
import concourse.tile as tile
from concourse import mybir
from concourse.bass import ts
from concourse.bass_utils import run_bass_kernel_spmd
from concourse.masks import make_identity

N_CORES = 8
B, T, E, D, A, C = 16, 2048, 1024, 1024, 512, 64
K_STATE, CONV_K = 2, 15
KK = 2 * CONV_K + 1  # 31
TPAD = T + 2 * CONV_K  # 2078
BL = B // N_CORES  # batches per core
NA = A // 128  # 4 a-chunks
NE = E // 128  # 8 e-chunks
TT = 512  # t-tile (free dim of proj matmuls)
NTT = T // TT  # 4 per batch
NTC = T // 128  # 16 t-chunks of 128 per batch

F32 = mybir.dt.float32
F32R = mybir.dt.float32r
BF16 = mybir.dt.bfloat16
AFT = mybir.ActivationFunctionType

_CACHE = {}

# tuning knobs (read at build time)
CFG = {
    "bufs_nat": 8,
    "bufs_natbf": 20,
    "bufs_encT": 16,
    "bufs_s": 10,
    "bufs_tpsum": 2,
    "bufs_ppsum": 2,
    "tr_bf16": True,   # PE transposes in bf16 (1 cyc/row) vs f32 (2 cyc/row)
}


def _declare_io(nc):
    t = lambda name, shape, dt, kind: nc.dram_tensor(name, shape, dt, kind=kind).ap()
    io = {
        "enc": t("enc", [BL, T, E], F32, "ExternalInput"),
        "attnpad": t("attnpad", [BL, K_STATE, TPAD], BF16, "ExternalInput"),
        "keep": t("keep", [BL, T], F32, "ExternalInput"),
        "wencT": t("wencT", [E, A], BF16, "ExternalInput"),
        "wfusedT": t("wfusedT", [64, A], BF16, "ExternalInput"),
        "wdecT": t("wdecT", [D, A], BF16, "ExternalInput"),
        "dechT": t("dechT", [D, BL], BF16, "ExternalInput"),
        "bencrow": t("bencrow", [1, A], BF16, "ExternalInput"),
        "woutcol": t("woutcol", [A, 1], BF16, "ExternalInput"),
        "c_out": t("c_out", [BL, E], F32, "ExternalOutput"),
        "w_out": t("w_out", [BL, T], F32, "ExternalOutput"),
    }
    return type("IO", (), io)


def _emit(ctx: ExitStack, tc: tile.TileContext, io, reps=1):
    nc = tc.nc

    const = ctx.enter_context(tc.tile_pool(name="const", bufs=1))
    spsum = ctx.enter_context(tc.tile_pool(name="spsum", bufs=2, space="PSUM"))
    tpsum = ctx.enter_context(tc.tile_pool(name="tpsum", bufs=CFG["bufs_tpsum"], space="PSUM"))
    ppsum = ctx.enter_context(tc.tile_pool(name="ppsum", bufs=CFG["bufs_ppsum"], space="PSUM"))
    cpsum = ctx.enter_context(tc.tile_pool(name="cpsum", bufs=2, space="PSUM"))
    natp = ctx.enter_context(tc.tile_pool(name="natp", bufs=CFG["bufs_nat"]))
    natbfp = ctx.enter_context(tc.tile_pool(name="natbfp", bufs=CFG["bufs_natbf"]))
    encTp = ctx.enter_context(tc.tile_pool(name="encTp", bufs=CFG["bufs_encT"]))
    sp = ctx.enter_context(tc.tile_pool(name="sp", bufs=CFG["bufs_s"]))
    shiftp = ctx.enter_context(tc.tile_pool(name="shiftp", bufs=2))
    misc = ctx.enter_context(tc.tile_pool(name="misc", bufs=2))

    # --- constants / weights ---
    ident_f32 = const.tile([128, 128], F32)
    make_identity(nc, ident_f32[:])
    ident = const.tile([128, 128], BF16)
    nc.vector.tensor_copy(ident[:], ident_f32[:])
    ones_mat = const.tile([128, 128], F32)
    nc.vector.memset(ones_mat[:], 1.0)
    ones_1xb = const.tile([1, BL], BF16)
    nc.vector.memset(ones_1xb[:], 1.0)

    wenc = const.tile([128, NE, A], BF16)
    nc.sync.dma_start(wenc[:], io.wencT.rearrange("(c p) a -> p c a", p=128))
    wfused = const.tile([64, A], BF16)
    nc.sync.dma_start(wfused[:], io.wfusedT)
    wdec = const.tile([128, NE, A], BF16)
    nc.sync.dma_start(wdec[:], io.wdecT.rearrange("(c p) a -> p c a", p=128))
    dech = const.tile([128, NE, BL], BF16)
    nc.sync.dma_start(dech[:], io.dechT.rearrange("(c p) b -> p c b", p=128))
    benc = const.tile([1, A], BF16)
    nc.sync.dma_start(benc[:], io.bencrow)
    wout = const.tile([128, NA], BF16)
    nc.sync.dma_start(wout[:], io.woutcol.rearrange("(c p) o -> p (c o)", p=128))
    keepc = const.tile([128, BL, NTC], F32)
    nc.sync.dma_start(keepc[:], io.keep.rearrange("b (c p) -> p b c", p=128))

    # --- per-(batch, a-chunk) tanh bias: dec_h @ W_dec.T + b_enc, in column form ---
    biasc = const.tile([128, NA, BL], F32)
    for ac in range(NA):
        ps = spsum.tile([128, BL], F32, tag="small")
        for ec in range(NE):
            nc.tensor.matmul(
                ps[:], wdec[:, ec, ts(ac, 128)], dech[:, ec, :],
                start=(ec == 0), stop=False,
            )
        nc.tensor.matmul(ps[:], benc[:, ts(ac, 128)], ones_1xb[:], start=False, stop=True)
        nc.vector.tensor_copy(biasc[:, ac, :], ps[:])

    # --- software-pipelined main loop over (batch, t-tile) ---
    # Per iteration i the PE stream is:
    #   transposes(i) | scores(i-1) | proj(i) | c-MMs(i-1)
    # so PE never waits on the ACT/DVE exp->mask->cast chain of its own tile.
    for r in range(reps):
        _emit_rep(nc, io, f"r{r}_", spsum, tpsum, ppsum, cpsum, natp, natbfp, encTp, sp,
                  shiftp, misc, ident, ident_f32, ones_mat, wenc, wfused, wout, keepc, biasc)


def _emit_rep(nc, io, pfx, spsum, tpsum, ppsum, cpsum, natp, natbfp, encTp, sp,
              shiftp, misc, ident, ident_f32, ones_mat, wenc, wfused, wout, keepc, biasc):
    batch_state = {}

    def new_batch(b):
        shifted = shiftp.tile([64, T], BF16, tag="shifted", name=f"{pfx}shifted_{b}")
        nc.gpsimd.memset(shifted[:], 0.0)
        for k in range(K_STATE):
            base = io.attnpad[b, k]
            win = bass.AP(base.tensor, base.offset, [[1, KK], [1, T]])
            nc.sync.dma_start(shifted[k * 32 : k * 32 + KK, :], win)
        st = {
            "shifted": shifted,
            "scol": misc.tile([128, NTC], F32, tag="scol", name=f"{pfx}scol_{b}"),
            "ucol": misc.tile([128, NTC], F32, tag="ucol", name=f"{pfx}ucol_{b}"),
            "ucol_bf": misc.tile([128, NTC], BF16, tag="ucolbf", name=f"{pfx}ucolbf_{b}"),
            "nat_bf": [],
        }
        batch_state[b] = st
        return st

    def emit_scores(p):
        b, tt, s_tiles = p
        st = batch_state[b]
        for q in range(4):
            ci = tt * 4 + q
            psu = spsum.tile([128, 1], F32, tag="small", name=f"{pfx}psu_{b}_{ci}")
            for ac in range(NA):
                nc.tensor.matmul(
                    psu[:], s_tiles[ac][:, ts(q, 128)], wout[:, ac : ac + 1],
                    start=(ac == 0), stop=(ac == NA - 1),
                )
            nc.vector.tensor_copy(st["scol"][:, ci : ci + 1], psu[:])

    def emit_batch_tail(b):
        st = batch_state[b]
        # u = exp(2*scores) * keep, once per batch; bf16 copy feeds the c-matmuls
        nc.scalar.activation(st["ucol"][:], st["scol"][:], AFT.Exp, scale=2.0)
        nc.vector.tensor_mul(st["ucol"][:], st["ucol"][:], keepc[:, b, :])
        nc.vector.tensor_copy(st["ucol_bf"][:], st["ucol"][:])
        # c = sum_t u[t] * enc[t, :] as a burst of 32 accumulating matmuls
        cps = [
            cpsum.tile([1, 512], F32, tag="cps", name=f"{pfx}cps_{b}_{eh}")
            for eh in range(2)
        ]
        for ci in range(NTC):
            for eh in range(2):
                nc.tensor.matmul(
                    cps[eh][:],
                    st["ucol_bf"][:, ci : ci + 1],
                    st["nat_bf"][ci][:, ts(eh, 512)],
                    start=(ci == 0), stop=(ci == NTC - 1),
                )
        usum = misc.tile([128, 1], F32, tag="usum", name=f"{pfx}usum_{b}")
        nc.vector.reduce_sum(usum[:], st["ucol"][:], axis=mybir.AxisListType.X)
        tot = spsum.tile([128, 1], F32, tag="small", name=f"{pfx}tot_{b}")
        nc.tensor.matmul(tot[:], ones_mat[:], usum[:], start=True, stop=True)
        inv = misc.tile([128, 1], F32, tag="inv", name=f"{pfx}inv_{b}")
        nc.vector.reciprocal(inv[:], tot[:])
        wcol = misc.tile([128, NTC], F32, tag="wcol", name=f"{pfx}wcol_{b}")
        nc.vector.tensor_scalar_mul(wcol[:], st["ucol"][:], inv[:])
        nc.sync.dma_start(io.w_out[b].rearrange("(c p) -> p c", p=128), wcol[:])
        crow = misc.tile([1, E], F32, tag="crow", name=f"{pfx}crow_{b}")
        for eh in range(2):
            nc.vector.tensor_scalar_mul(crow[:, ts(eh, 512)], cps[eh][:], inv[0:1, :])
        nc.sync.dma_start(io.c_out[b].unsqueeze(0), crow[:])

    pending = None
    for b in range(BL):
        st = new_batch(b)
        for tt in range(NTT):
            # stage 1: load + cast + transposes of tile i
            nat = []
            nat_bf = []
            for q in range(4):
                nt = natp.tile([128, E], F32, tag="nat", name=f"{pfx}nat_{b}_{tt}_{q}")
                nc.sync.dma_start(
                    nt[:], io.enc[b, tt * TT + q * 128 : tt * TT + (q + 1) * 128, :]
                )
                nat.append(nt)
                nb = natbfp.tile([128, E], BF16, tag="natbf", name=f"{pfx}natbf_{b}_{tt}_{q}")
                nc.vector.tensor_copy(nb[:], nt[:])
                nat_bf.append(nb)
                st["nat_bf"].append(nb)
            encT = []
            for ec in range(NE):
                trdt = BF16 if CFG["tr_bf16"] else F32
                pst = tpsum.tile([128, TT], trdt, tag="tr", name=f"{pfx}tr_{b}_{tt}_{ec}")
                for q in range(4):
                    src_t = nat_bf[q] if CFG["tr_bf16"] else nat[q]
                    idn = ident if CFG["tr_bf16"] else ident_f32
                    nc.tensor.transpose(
                        pst[:, ts(q, 128)], src_t[:, ts(ec, 128)], idn[:]
                    )
                eb = encTp.tile([128, TT], BF16, tag="encT", name=f"{pfx}encT_{b}_{tt}_{ec}")
                nc.vector.tensor_copy(eb[:], pst[:])
                encT.append(eb)
            # stage 2: scores of tile i-1 (PE work is tiny; feeds ACT/DVE chain)
            if pending is not None:
                emit_scores(pending)
            # stage 3: proj + tanh of tile i
            s_tiles = []
            for ac in range(NA):
                ps = ppsum.tile([128, TT], F32, tag="proj", name=f"{pfx}proj_{b}_{tt}_{ac}")
                for ec in range(NE):
                    nc.tensor.matmul(
                        ps[:], wenc[:, ec, ts(ac, 128)], encT[ec][:],
                        start=(ec == 0), stop=False,
                    )
                nc.tensor.matmul(
                    ps[:], wfused[:, ts(ac, 128)], st["shifted"][:, ts(tt, TT)],
                    start=False, stop=True,
                )
                s = sp.tile([128, TT], BF16, tag="s", name=f"{pfx}s_{b}_{tt}_{ac}")
                nc.scalar.activation(s[:], ps[:], AFT.Tanh, bias=biasc[:, ac, b : b + 1])
                s_tiles.append(s)
            # stage 4: batch tail of tile i-1's batch if it was that batch's last
            if pending is not None:
                pb, ptt = pending[0], pending[1]
                if ptt == NTT - 1:
                    emit_batch_tail(pb)
            pending = (b, tt, s_tiles)
    # flush
    emit_scores(pending)
    emit_batch_tail(pending[0])


def build(reps=1):
    key = ("nc", reps)
    if key in _CACHE:
        return _CACHE[key]
    nc = bacc.Bacc("TRN2", target_bir_lowering=False, debug=False, num_devices=N_CORES)
    io = _declare_io(nc)
    with tile.TileContext(nc) as tc, ExitStack() as ctx:
        _emit(ctx, tc, io, reps=reps)
    nc.compile()
    _CACHE[key] = nc
    return nc


def prep_in_maps(inputs):
    enc = np.ascontiguousarray(np.asarray(inputs["encoder_out"], dtype=np.float32))
    mask = np.asarray(inputs["encoder_padding_mask"])
    dec_h = np.asarray(inputs["decoder_h"], dtype=np.float32)
    attn_state = np.asarray(inputs["attn_state"], dtype=np.float32)
    W_enc = np.asarray(inputs["W_enc"], dtype=np.float32)
    b_enc = np.asarray(inputs["b_enc"], dtype=np.float32)
    W_dec = np.asarray(inputs["W_dec"], dtype=np.float32)
    W_attn = np.asarray(inputs["W_attn"], dtype=np.float32)
    W_conv = np.asarray(inputs["W_conv"], dtype=np.float32)
    W_out = np.asarray(inputs["W_out"], dtype=np.float32)

    bf = ml_dtypes.bfloat16
    keep = 1.0 - mask.astype(np.float32)  # [B, T]
    attnpad = np.zeros((B, K_STATE, TPAD), dtype=np.float32)
    attnpad[:, :, CONV_K : CONV_K + T] = attn_state
    attnpad = attnpad.astype(bf)
    # fused conv+attn projection: [A, C] @ [C, K*KK] -> [A, 62] -> pad to 64 rows
    wfu = (W_attn @ W_conv.reshape(C, K_STATE * KK)).reshape(A, K_STATE, KK)
    wfusedT = np.zeros((64, A), dtype=np.float32)
    for k in range(K_STATE):
        wfusedT[k * 32 : k * 32 + KK, :] = wfu[:, k, :].T
    wfusedT = np.ascontiguousarray(wfusedT).astype(bf)
    wencT = np.ascontiguousarray(W_enc.T).astype(bf)
    wdecT = np.ascontiguousarray(W_dec.T).astype(bf)
    bencrow = b_enc.reshape(1, A).astype(bf)
    woutcol = np.ascontiguousarray(W_out.reshape(1, A).T).astype(bf)

    in_maps = []
    for i in range(N_CORES):
        sl = slice(i * BL, (i + 1) * BL)
        in_maps.append(
            {
                "enc": enc[sl],
                "attnpad": np.ascontiguousarray(attnpad[sl]),
                "keep": np.ascontiguousarray(keep[sl]),
                "wencT": wencT,
                "wfusedT": wfusedT,
                "wdecT": wdecT,
                "dechT": np.ascontiguousarray(dec_h[sl].T).astype(bf),
                "bencrow": bencrow,
                "woutcol": woutcol,
            }
        )
    return in_maps


def kernel(**inputs):
    nc = build()
    in_maps = prep_in_maps(inputs)
    res = run_bass_kernel_spmd(nc, in_maps, list(range(N_CORES)))
    c = np.concatenate([res.results[i]["c_out"] for i in range(N_CORES)], axis=0)
    w = np.concatenate([res.results[i]["w_out"] for i in range(N_CORES)], axis=0)
    return c.astype(np.float32), w.astype(np.float32)
